# revision 12
# baseline (speedup 1.0000x reference)
"""Inverse STFT (nn_InverseSTFT) as a Bass/Tile kernel on 8 TRN2 NeuronCores.

Math
----
Reference: per batch, fold conjugate symmetry into a real K=1024 basis so
  ytmp[w, t] = sum_k basis[k, w] * x[k, t],   w = 0..1023
  y = overlap_add(ytmp, hop=256) / wss, trim first 2 segments.

Radix-2 split: every folded-basis row k corresponds to an original
frequency f(k) and satisfies  basis[k, w+512] = (-1)^{f(k)} basis[k, w].
Splitting rows by parity of f:
  Se[w2, t] = sum_{k even} basis[k, w2] x[k, t]     (K=512, w2=0..511)
  So[w2, t] = sum_{k odd } basis[k, w2] x[k, t]
  ytmp[w2]       = Se + So
  ytmp[w2 + 512] = Se - So
This HALVES the PE matmul work vs the dense K=1024 transform.

With w = 256*j + r and u' = s - 2 (trimmed segments), overlap-add becomes
column-shifted, partition-aligned combines of Se/So (j=0,1 from Se+So,
j=2,3 from Se-So at shifted frames):
  y[u', r] = Se0[r, u'+3] + Se0[r, u'+1] + Se1[r, u'+2] + Se1[r, u']
           + So0[r, u'+3] - So0[r, u'+1] + So1[r, u'+2] - So1[r, u']
(tc = frame + 1; Sx0 = w2-tile 256*0+128rt, Sx1 = w2-tile 256*1+128rt.)
These are 7 DVE tensor_add/sub ops per output tile, overlapped with the
next tile's matmuls.

Window-sum normalization = 0.25 folded into basis; edge segments get a
per-column fixup (u'=0 -> 4/3, u'=1998 -> 4/3, 1999 -> 2, 2000 -> 4).
Device output is [r-partitions, u'-columns]; host transposes (free).

Sharding: pure data parallel, 2 batches per core.
"""

import numpy as np

import concourse.bass as bass
import concourse.mybir as mybir
from concourse.tile import TileContext
from concourse import bacc, bass_utils

N_FFT = 1024
HOP = 256
B = 16
NFREQ = 513
T = 2000
NCORES = 8
NB = B // NCORES          # batches per core
KC = 4                    # K chunks of 128 per parity half (K_half = 512)
XW = 2048                 # frame t lives at column t+1; t = -1..2046
OUT_SEGS = 2001           # segments s = 2..2002  (u' = 0..2000)
OUT_LEN = OUT_SEGS * HOP  # 512256
NRT = 2                   # r-tiles of 128 (r = 0..255)
NWT = 4                   # w2-tiles of 128 (w2 = 0..511)
CHUNKS = [(0, 512), (512, 512), (1024, 512), (1536, OUT_SEGS - 1536)]

F32 = mybir.dt.float32

import os as _os

USE_BF16 = _os.environ.get("ISTFT_BF16", "1") == "1"
DT_IN = mybir.dt.bfloat16 if USE_BF16 else F32

import ml_dtypes

NP_IN = ml_dtypes.bfloat16 if USE_BF16 else np.float32


def _make_basis() -> np.ndarray:
    """(1024, 1024) folded basis, matching reference's float32 angle math."""
    f = np.arange(N_FFT, dtype=np.float32)
    w = np.arange(N_FFT, dtype=np.float32)
    a32 = np.float32(2.0 * np.pi / N_FFT)
    t1 = (a32 * f).astype(np.float32)
    ang = (t1[:, None] * w[None, :]).astype(np.float32)
    reb = (np.cos(ang).astype(np.float32) / np.float32(N_FFT)).astype(np.float32)
    imb = (-np.sin(ang).astype(np.float32) / np.float32(N_FFT)).astype(np.float32)
    A = np.empty((NFREQ, N_FFT), np.float32)
    A[0] = reb[0]
    A[512] = reb[512]
    A[1:512] = reb[1:512] + reb[1023:512:-1]
    Bm = (imb[1:512] - imb[1023:512:-1]).astype(np.float32)
    return np.concatenate([A, Bm], axis=0)


# original frequency f of each folded-basis row (rows 0..512 = cos f,
# rows 513..1023 = sin f, f = 1..511)
_F_OF_ROW = np.concatenate([np.arange(NFREQ), np.arange(1, 512)])
_EVEN_ROWS = np.where(_F_OF_ROW % 2 == 0)[0]   # 512 rows
_ODD_ROWS = np.where(_F_OF_ROW % 2 == 1)[0]    # 512 rows


def _prep_x(stft: np.ndarray) -> np.ndarray:
    """(16,513,2000,2) f32 -> (16, 2, KC, 128, XW); [:,0]=even-f, [:,1]=odd-f."""
    re = stft[:, :, :, 0]                  # (B, 513, T)
    im = stft[:, 1:512, :, 1]              # (B, 511, T)
    xk = np.concatenate([re, im], axis=1)  # (B, 1024, T)
    X = np.zeros((B, 2, 512, XW), np.float32)
    X[:, 0, :, 1 : 1 + T] = xk[:, _EVEN_ROWS]
    X[:, 1, :, 1 : 1 + T] = xk[:, _ODD_ROWS]
    return np.ascontiguousarray(X.reshape(B, 2, KC, 128, XW))


def _prep_basis() -> np.ndarray:
    """(2, KC, 128, 512): parity-split rows, first 512 w cols, x0.25 wss."""
    bas = _make_basis() * np.float32(0.25)
    C = np.stack([bas[_EVEN_ROWS, :512], bas[_ODD_ROWS, :512]])
    return np.ascontiguousarray(C.reshape(2, KC, 128, 512))


# (chunk index, column within chunk, scale) edge fixups on top of the 0.25
# folded into basis: u'=0 has 3 overlapping frames, 1998 -> 3, 1999 -> 2,
# 2000 -> 1.
EDGE_FIX = [
    (0, 0, 4.0 / 3.0),
    (3, 1998 - 1536, 4.0 / 3.0),
    (3, 1999 - 1536, 2.0),
    (3, 2000 - 1536, 4.0),
]


def _build_nc() -> bass.Bass:
    nc = bacc.Bacc()
    x_in = nc.dram_tensor("x_in", [NB, 2, KC, 128, XW], DT_IN, kind="ExternalInput")
    basis_in = nc.dram_tensor("basis_in", [2, KC, 128, 512], DT_IN, kind="ExternalInput")
    out = nc.dram_tensor("out", [NB, NRT, 128, OUT_SEGS], DT_IN, kind="ExternalOutput")

    with TileContext(nc) as tc:
        with (
            tc.tile_pool(name="xp", bufs=1) as x_pool,
            tc.tile_pool(name="bp", bufs=1) as b_pool,
            tc.tile_pool(name="st", bufs=1) as st_pool,
            tc.tile_pool(name="ev", bufs=4) as ev_pool,
            tc.tile_pool(name="ps", bufs=6, space="PSUM") as psum_pool,
            tc.tile_pool(name="wu", bufs=1, space="PSUM") as wu_pool,
        ):
            # PE warm-up: ~64 dummy matmuls on a never-DMA'd scratch tile.
            # They have no semaphore deps, so they run during the ~10us
            # input-DMA window and hold the HAM clock gate at full rate
            # (2.4 GHz) by the time the real matmuls start; results are
            # discarded.
            wu_x = x_pool.tile([128, 256], DT_IN, name="wu_x", tag="wu_x")
            wu_ps = wu_pool.tile([128, 256], F32, name="wu_ps", tag="wu_ps")
            nc.scalar.memzero(wu_x[:, :])
            for _ in range(64):
                nc.tensor.matmul(
                    wu_ps[:, :], wu_x[:, :128], wu_x[:, :], start=True, stop=True
                )
            # basis chunks issue first on the Sync HWDGE queues (the first
            # matmul's stationary operand is the critical path); x goes via
            # GpSimd so the two DMA instruction streams issue in parallel.
            c_sb = [[None] * KC for _ in range(2)]
            for S in range(2):
                for kc in range(KC):
                    bt = b_pool.tile(
                        [128, 512], DT_IN, name=f"bas{S}_{kc}", tag=f"bas{S}_{kc}"
                    )
                    nc.sync.dma_start(bt[:, :], basis_in[S, kc])
                    c_sb[S][kc] = bt

            x_sb = [[[None] * KC for _ in range(2)] for _ in range(NB)]
            for b in range(NB):
                for S in range(2):
                    for kc in range(KC):
                        xt = x_pool.tile(
                            [128, XW], DT_IN, name=f"x{b}_{S}{kc}", tag=f"x{b}_{S}{kc}"
                        )
                        x_sb[b][S][kc] = xt
            # column-split DMAs: all first halves before any second half,
            # so the first matmuls (tc chunks 0-1) start as early as possible
            for half in range(2):
                c0, c1 = (0, XW // 2) if half == 0 else (XW // 2, XW)
                for b in range(NB):
                    for S in range(2):
                        for kc in range(KC):
                            nc.gpsimd.dma_start(
                                x_sb[b][S][kc][:, c0:c1], x_in[b, S, kc, :, c0:c1]
                            )

            # Se/So accumulators in SBUF (bf16), [128 w2-part, 2048 tc]
            st_sb = [[[None] * NWT for _ in range(2)] for _ in range(NB)]
            for b in range(NB):
                for S in range(2):
                    for wt in range(NWT):
                        st_sb[b][S][wt] = st_pool.tile(
                            [128, XW], DT_IN, name=f"st{b}_{S}{wt}", tag=f"st{b}_{S}{wt}"
                        )

            def stage1(b, S, wt):
                for tcn in range(4):
                    # combine only reads tc <= 2003: trim the last chunk
                    ncols = 512 if tcn < 3 else 2004 - 1536
                    ps = psum_pool.tile([128, 512], F32, name="ps", tag="ps")
                    for kc in range(KC):
                        nc.tensor.matmul(
                            ps[:, :ncols],
                            c_sb[S][kc][:, 128 * wt : 128 * wt + 128],
                            x_sb[b][S][kc][:, 512 * tcn : 512 * tcn + ncols],
                            start=(kc == 0),
                            stop=(kc == KC - 1),
                        )
                    nc.scalar.copy(
                        st_sb[b][S][wt][:, 512 * tcn : 512 * tcn + ncols],
                        ps[:, :ncols],
                    )

            def combine(b, rt):
                e0, e1 = st_sb[b][0][rt], st_sb[b][0][2 + rt]
                o0, o1 = st_sb[b][1][rt], st_sb[b][1][2 + rt]
                for ci, (c0, ncols) in enumerate(CHUNKS):
                    # bf16 accumulator: all-16-bit stride-1 streams run the
                    # DVE in 2x mode (the fp32-acc version was DVE-bound)
                    acc = ev_pool.tile([128, 512], DT_IN, name="ev", tag="ev")
                    a = acc[:, :ncols]
                    nc.vector.tensor_add(
                        a, e0[:, c0 + 3 : c0 + 3 + ncols], e0[:, c0 + 1 : c0 + 1 + ncols]
                    )
                    nc.vector.tensor_add(a, a, e1[:, c0 + 2 : c0 + 2 + ncols])
                    nc.vector.tensor_add(a, a, e1[:, c0 : c0 + ncols])
                    nc.vector.tensor_add(a, a, o0[:, c0 + 3 : c0 + 3 + ncols])
                    nc.vector.tensor_sub(a, a, o0[:, c0 + 1 : c0 + 1 + ncols])
                    nc.vector.tensor_add(a, a, o1[:, c0 + 2 : c0 + 2 + ncols])
                    nc.vector.tensor_sub(a, a, o1[:, c0 : c0 + ncols])
                    for fci, fcol, fsc in EDGE_FIX:
                        if fci == ci:
                            nc.scalar.mul(
                                acc[:, fcol : fcol + 1],
                                acc[:, fcol : fcol + 1],
                                float(fsc),
                            )
                    nc.sync.dma_start(out[b, rt, :, c0 : c0 + ncols], acc[:, :ncols])

            # interleave: emit the two w2-tiles each r-tile needs, then its
            # combine, so DVE combines overlap the next r-tile's matmuls.
            for b in range(NB):
                for rt in range(NRT):
                    for S in range(2):
                        stage1(b, S, rt)
                        stage1(b, S, 2 + rt)
                    combine(b, rt)
    nc.finalize()
    return nc


def _run(inputs: dict, trace: bool = False):
    stft = np.asarray(inputs["stft_matrix"], dtype=np.float32)
    X = np.ascontiguousarray(_prep_x(stft).astype(NP_IN))
    basis = np.ascontiguousarray(_prep_basis().astype(NP_IN))

    in_maps = [
        {"x_in": X[NB * c : NB * (c + 1)], "basis_in": basis}
        for c in range(NCORES)
    ]
    nc = _build_nc()
    res = bass_utils.run_bass_kernel_spmd(
        nc, in_maps, core_ids=list(range(NCORES)), trace=trace
    )
    # device output is [NB, rt, r_part, u']; host transpose to [NB, u', r]
    out = np.concatenate(
        [
            np.ascontiguousarray(
                np.transpose(
                    res.results[c]["out"].astype(np.float32), (0, 3, 1, 2)
                )
            ).reshape(NB, OUT_LEN)
            for c in range(NCORES)
        ],
        axis=0,
    )
    return out, res


def kernel(**inputs) -> np.ndarray:
    out, _ = _run(inputs, trace=False)
    return out


# revision 14
# speedup vs baseline: 1.0461x; 1.0461x over previous
"""Inverse STFT (nn_InverseSTFT) as a Bass/Tile kernel on 8 TRN2 NeuronCores.

Math
----
Reference: per batch, fold conjugate symmetry into a real K=1024 basis so
  ytmp[w, t] = sum_k basis[k, w] * x[k, t],   w = 0..1023
  y = overlap_add(ytmp, hop=256) / wss, trim first 2 segments.

Radix-2 split: every folded-basis row k corresponds to an original
frequency f(k) and satisfies  basis[k, w+512] = (-1)^{f(k)} basis[k, w].
Splitting rows by parity of f:
  Se[w2, t] = sum_{k even} basis[k, w2] x[k, t]     (K=512, w2=0..511)
  So[w2, t] = sum_{k odd } basis[k, w2] x[k, t]
  ytmp[w2]       = Se + So
  ytmp[w2 + 512] = Se - So
This HALVES the PE matmul work vs the dense K=1024 transform.

With w = 256*j + r and u' = s - 2 (trimmed segments), overlap-add becomes
column-shifted, partition-aligned combines of Se/So (j=0,1 from Se+So,
j=2,3 from Se-So at shifted frames):
  y[u', r] = Se0[r, u'+3] + Se0[r, u'+1] + Se1[r, u'+2] + Se1[r, u']
           + So0[r, u'+3] - So0[r, u'+1] + So1[r, u'+2] - So1[r, u']
(tc = frame + 1; Sx0 = w2-tile 256*0+128rt, Sx1 = w2-tile 256*1+128rt.)
These are 7 DVE tensor_add/sub ops per output tile, overlapped with the
next tile's matmuls.

Window-sum normalization = 0.25 folded into basis; edge segments get a
per-column fixup (u'=0 -> 4/3, u'=1998 -> 4/3, 1999 -> 2, 2000 -> 4).
Device output is [r-partitions, u'-columns]; host transposes (free).

Sharding: pure data parallel, 2 batches per core.
"""

import numpy as np

import concourse.bass as bass
import concourse.mybir as mybir
from concourse.tile import TileContext
from concourse import bacc, bass_utils

N_FFT = 1024
HOP = 256
B = 16
NFREQ = 513
T = 2000
NCORES = 8
NB = B // NCORES          # batches per core
KC = 4                    # K chunks of 128 per parity half (K_half = 512)
XW = 2048                 # frame t lives at column t+1; t = -1..2046
OUT_SEGS = 2001           # segments s = 2..2002  (u' = 0..2000)
OUT_LEN = OUT_SEGS * HOP  # 512256
NRT = 2                   # r-tiles of 128 (r = 0..255)
NWT = 4                   # w2-tiles of 128 (w2 = 0..511)
CHUNKS = [(0, 512), (512, 512), (1024, 512), (1536, OUT_SEGS - 1536)]

F32 = mybir.dt.float32

import os as _os

USE_BF16 = _os.environ.get("ISTFT_BF16", "1") == "1"
DT_IN = mybir.dt.bfloat16 if USE_BF16 else F32

import ml_dtypes

NP_IN = ml_dtypes.bfloat16 if USE_BF16 else np.float32


def _make_basis() -> np.ndarray:
    """(1024, 1024) folded basis, matching reference's float32 angle math."""
    f = np.arange(N_FFT, dtype=np.float32)
    w = np.arange(N_FFT, dtype=np.float32)
    a32 = np.float32(2.0 * np.pi / N_FFT)
    t1 = (a32 * f).astype(np.float32)
    ang = (t1[:, None] * w[None, :]).astype(np.float32)
    reb = (np.cos(ang).astype(np.float32) / np.float32(N_FFT)).astype(np.float32)
    imb = (-np.sin(ang).astype(np.float32) / np.float32(N_FFT)).astype(np.float32)
    A = np.empty((NFREQ, N_FFT), np.float32)
    A[0] = reb[0]
    A[512] = reb[512]
    A[1:512] = reb[1:512] + reb[1023:512:-1]
    Bm = (imb[1:512] - imb[1023:512:-1]).astype(np.float32)
    return np.concatenate([A, Bm], axis=0)


# original frequency f of each folded-basis row (rows 0..512 = cos f,
# rows 513..1023 = sin f, f = 1..511)
_F_OF_ROW = np.concatenate([np.arange(NFREQ), np.arange(1, 512)])
_EVEN_ROWS = np.where(_F_OF_ROW % 2 == 0)[0]   # 512 rows
_ODD_ROWS = np.where(_F_OF_ROW % 2 == 1)[0]    # 512 rows


def _prep_x(stft: np.ndarray) -> np.ndarray:
    """(16,513,2000,2) f32 -> (16, 2, KC, 128, XW); [:,0]=even-f, [:,1]=odd-f."""
    re = stft[:, :, :, 0]                  # (B, 513, T)
    im = stft[:, 1:512, :, 1]              # (B, 511, T)
    xk = np.concatenate([re, im], axis=1)  # (B, 1024, T)
    X = np.zeros((B, 2, 512, XW), np.float32)
    X[:, 0, :, 1 : 1 + T] = xk[:, _EVEN_ROWS]
    X[:, 1, :, 1 : 1 + T] = xk[:, _ODD_ROWS]
    return np.ascontiguousarray(X.reshape(B, 2, KC, 128, XW))


def _prep_basis() -> np.ndarray:
    """(2, KC, 128, 512): parity-split rows, first 512 w cols, x0.25 wss."""
    bas = _make_basis() * np.float32(0.25)
    C = np.stack([bas[_EVEN_ROWS, :512], bas[_ODD_ROWS, :512]])
    return np.ascontiguousarray(C.reshape(2, KC, 128, 512))


# (chunk index, column within chunk, scale) edge fixups on top of the 0.25
# folded into basis: u'=0 has 3 overlapping frames, 1998 -> 3, 1999 -> 2,
# 2000 -> 1.
EDGE_FIX = [
    (0, 0, 4.0 / 3.0),
    (3, 1998 - 1536, 4.0 / 3.0),
    (3, 1999 - 1536, 2.0),
    (3, 2000 - 1536, 4.0),
]


def _build_nc() -> bass.Bass:
    nc = bacc.Bacc()
    x_in = nc.dram_tensor("x_in", [NB, 2, KC, 128, XW], DT_IN, kind="ExternalInput")
    basis_in = nc.dram_tensor("basis_in", [2, KC, 128, 512], DT_IN, kind="ExternalInput")
    out = nc.dram_tensor("out", [NB, NRT, 128, OUT_SEGS], DT_IN, kind="ExternalOutput")

    with TileContext(nc) as tc:
        with (
            tc.tile_pool(name="xp", bufs=1) as x_pool,
            tc.tile_pool(name="bp", bufs=1) as b_pool,
            tc.tile_pool(name="st", bufs=1) as st_pool,
            tc.tile_pool(name="ev", bufs=4) as ev_pool,
            tc.tile_pool(name="ps", bufs=6, space="PSUM") as psum_pool,
            tc.tile_pool(name="wu", bufs=1, space="PSUM") as wu_pool,
        ):
            # PE warm-up: ~64 dummy matmuls on a never-DMA'd scratch tile.
            # They have no semaphore deps, so they run during the ~10us
            # input-DMA window and hold the HAM clock gate at full rate
            # (2.4 GHz) by the time the real matmuls start; results are
            # discarded.
            wu_x = x_pool.tile([128, 256], DT_IN, name="wu_x", tag="wu_x")
            wu_ps = wu_pool.tile([128, 256], F32, name="wu_ps", tag="wu_ps")
            nc.scalar.memzero(wu_x[:, :])
            for _ in range(28):
                nc.tensor.matmul(
                    wu_ps[:, :], wu_x[:, :128], wu_x[:, :], start=True, stop=True
                )
            # basis chunks issue first on the Sync HWDGE queues (the first
            # matmul's stationary operand is the critical path); x goes via
            # GpSimd so the two DMA instruction streams issue in parallel.
            c_sb = [[None] * KC for _ in range(2)]
            for S in range(2):
                for kc in range(KC):
                    bt = b_pool.tile(
                        [128, 512], DT_IN, name=f"bas{S}_{kc}", tag=f"bas{S}_{kc}"
                    )
                    nc.sync.dma_start(bt[:, :], basis_in[S, kc])
                    c_sb[S][kc] = bt

            x_sb = [[[None] * KC for _ in range(2)] for _ in range(NB)]
            for b in range(NB):
                for S in range(2):
                    for kc in range(KC):
                        xt = x_pool.tile(
                            [128, XW], DT_IN, name=f"x{b}_{S}{kc}", tag=f"x{b}_{S}{kc}"
                        )
                        x_sb[b][S][kc] = xt
            # column-split DMAs, per batch: each batch's first halves land
            # before its second halves so tc chunks 0-1 start early, but
            # batch b's tails are not delayed behind batch b+1's heads
            for b in range(NB):
                for half in range(2):
                    c0, c1 = (0, XW // 2) if half == 0 else (XW // 2, XW)
                    for S in range(2):
                        for kc in range(KC):
                            nc.gpsimd.dma_start(
                                x_sb[b][S][kc][:, c0:c1], x_in[b, S, kc, :, c0:c1]
                            )

            # Se/So accumulators in SBUF (bf16), [128 w2-part, 2048 tc]
            st_sb = [[[None] * NWT for _ in range(2)] for _ in range(NB)]
            for b in range(NB):
                for S in range(2):
                    for wt in range(NWT):
                        st_sb[b][S][wt] = st_pool.tile(
                            [128, XW], DT_IN, name=f"st{b}_{S}{wt}", tag=f"st{b}_{S}{wt}"
                        )

            def stage1(b, S, wt):
                for tcn in range(4):
                    # combine only reads tc <= 2003: trim the last chunk
                    ncols = 512 if tcn < 3 else 2004 - 1536
                    ps = psum_pool.tile([128, 512], F32, name="ps", tag="ps")
                    for kc in range(KC):
                        nc.tensor.matmul(
                            ps[:, :ncols],
                            c_sb[S][kc][:, 128 * wt : 128 * wt + 128],
                            x_sb[b][S][kc][:, 512 * tcn : 512 * tcn + ncols],
                            start=(kc == 0),
                            stop=(kc == KC - 1),
                        )
                    nc.scalar.copy(
                        st_sb[b][S][wt][:, 512 * tcn : 512 * tcn + ncols],
                        ps[:, :ncols],
                    )

            def combine(b, rt):
                e0, e1 = st_sb[b][0][rt], st_sb[b][0][2 + rt]
                o0, o1 = st_sb[b][1][rt], st_sb[b][1][2 + rt]
                for ci, (c0, ncols) in enumerate(CHUNKS):
                    # bf16 accumulator: all-16-bit stride-1 streams run the
                    # DVE in 2x mode (the fp32-acc version was DVE-bound)
                    acc = ev_pool.tile([128, 512], DT_IN, name="ev", tag="ev")
                    a = acc[:, :ncols]
                    nc.vector.tensor_add(
                        a, e0[:, c0 + 3 : c0 + 3 + ncols], e0[:, c0 + 1 : c0 + 1 + ncols]
                    )
                    nc.vector.tensor_add(a, a, e1[:, c0 + 2 : c0 + 2 + ncols])
                    nc.vector.tensor_add(a, a, e1[:, c0 : c0 + ncols])
                    nc.vector.tensor_add(a, a, o0[:, c0 + 3 : c0 + 3 + ncols])
                    nc.vector.tensor_sub(a, a, o0[:, c0 + 1 : c0 + 1 + ncols])
                    nc.vector.tensor_add(a, a, o1[:, c0 + 2 : c0 + 2 + ncols])
                    nc.vector.tensor_sub(a, a, o1[:, c0 : c0 + ncols])
                    for fci, fcol, fsc in EDGE_FIX:
                        if fci == ci:
                            nc.scalar.mul(
                                acc[:, fcol : fcol + 1],
                                acc[:, fcol : fcol + 1],
                                float(fsc),
                            )
                    nc.sync.dma_start(out[b, rt, :, c0 : c0 + ncols], acc[:, :ncols])

            # interleave: emit the two w2-tiles each r-tile needs, then its
            # combine, so DVE combines overlap the next r-tile's matmuls.
            for b in range(NB):
                for rt in range(NRT):
                    for S in range(2):
                        stage1(b, S, rt)
                        stage1(b, S, 2 + rt)
                    combine(b, rt)
    nc.finalize()
    return nc


def _run(inputs: dict, trace: bool = False):
    stft = np.asarray(inputs["stft_matrix"], dtype=np.float32)
    X = np.ascontiguousarray(_prep_x(stft).astype(NP_IN))
    basis = np.ascontiguousarray(_prep_basis().astype(NP_IN))

    in_maps = [
        {"x_in": X[NB * c : NB * (c + 1)], "basis_in": basis}
        for c in range(NCORES)
    ]
    nc = _build_nc()
    res = bass_utils.run_bass_kernel_spmd(
        nc, in_maps, core_ids=list(range(NCORES)), trace=trace
    )
    # device output is [NB, rt, r_part, u']; host transpose to [NB, u', r]
    out = np.concatenate(
        [
            np.ascontiguousarray(
                np.transpose(
                    res.results[c]["out"].astype(np.float32), (0, 3, 1, 2)
                )
            ).reshape(NB, OUT_LEN)
            for c in range(NCORES)
        ],
        axis=0,
    )
    return out, res


def kernel(**inputs) -> np.ndarray:
    out, _ = _run(inputs, trace=False)
    return out
